# revision 5
# baseline (speedup 1.0000x reference)
"""DenseGATConv Trainium2 kernel v2 (8 NeuronCores, SPMD, column-sharded).

Same math as v1 (see kernel.py docstring):
    u_i = exp(0.2 a_src_i), e_i = exp(a_src_i), q_j = exp(0.8 a_dst_j)
    M[i,j] = adj[i,j] * max(e_i q_j, u_i)
    out[j,:] = (M^T h)[j,:] / colsum(M)[j] + bias.

v2 engine/schedule changes (driven by timeline-sim engine occupancy):
  - adj DMA'd in 1MB quad-tile chunks interleaved with the xT chunks on the
    SP queue so the adjacency stream is continuous and the tail chunk small.
  - mask-mult (t2 * adj) done as one tensor_tensor per 4-tile span: fewer
    DVE instructions, same 2x DVE mode, lower per-op overhead.
  - h/a_src PSUM->SBUF copies moved from ACT to the idle Pool (gpsimd)
    engine so ACT never serializes the h pipeline (exp groups + q_rep only).
  - a_src fused as a 129th column of the W matmul (no separate PE columns).
  - a_dst path in fp8 (q_j errors cancel between numerator and column sum).
  - numT exported fp16 (den stays f32); halves the output DMA.
"""

import numpy as np
import ml_dtypes
from contextlib import ExitStack

import concourse.bass as bass
import concourse.bacc as bacc
import concourse.tile as tile
from concourse import mybir
from concourse.bass_utils import run_bass_kernel_spmd

F32 = mybir.dt.float32
F16 = mybir.dt.float16
F8 = mybir.dt.float8e4
ALU = mybir.AluOpType
ACTF = mybir.ActivationFunctionType

N, C_IN, C_OUT = 8192, 256, 128
NCORES = 8
JB = N // NCORES          # 1024 destination columns per core
NT = N // 128             # 64 i-tiles
QUAD = 4                  # i-tiles per adj DMA chunk / per mask-mult op
NQ = NT // QUAD           # 16 quads
GRP = 4                   # a_src exp-group size (i-tiles)
XB = 16                   # i-tiles per xT chunk
NXC = NT // XB            # 4 xT chunks

_nc_cache = {}


def _emit_body(tc, nc, ctx, tensors):
    (xT_in, xTloc_in, adj_in, W_in, att_rep_in,
     numT_out, den_out) = tensors

    adj_r = adj_in.rearrange("(c a p) j -> c p a j", a=QUAD, p=128)

    const = ctx.enter_context(tc.tile_pool(name="const", bufs=1))
    xt_pool = ctx.enter_context(tc.tile_pool(name="xt", bufs=NXC))
    h_pool = ctx.enter_context(tc.tile_pool(name="h", bufs=NT))
    scratch = ctx.enter_context(tc.tile_pool(name="scr", bufs=2))
    adj_pool = ctx.enter_context(tc.tile_pool(name="adj", bufs=8))
    t2_pool = ctx.enter_context(tc.tile_pool(name="t2", bufs=3))
    m_pool = ctx.enter_context(tc.tile_pool(name="m", bufs=4))
    dsum_pool = ctx.enter_context(tc.tile_pool(name="dsum", bufs=2))
    ps_h = ctx.enter_context(tc.tile_pool(name="psh", bufs=2, space="PSUM"))
    ps_acc = ctx.enter_context(tc.tile_pool(name="psacc", bufs=1, space="PSUM"))
    ps_pre = ctx.enter_context(tc.tile_pool(name="pspre", bufs=1, space="PSUM"))

    # ---- front-loaded DMAs (SP queue, program order == stream order) ----
    # small constants first, then xTloc (a_dst path), then xc0, adj0, xc1,
    # adj1, ... so q_rep and the first h/exp groups are ready when the first
    # adjacency quads land.
    W_sb = const.tile([128, 258], F16, name="W_sb")      # [k*129 .. ] cols,
    W_view = W_sb[:].rearrange("p (two c) -> p two c", two=2)[:, :, 0:128]
    nc.sync.dma_start(W_view, W_in[:].rearrange("p (two c) -> p two c", two=2))

    xc = [xt_pool.tile([128, 2 * XB * 128], F16, tag="xtc", name=f"xc{cx}")
          for cx in range(NXC)]

    def emit_xc_dma(cx):
        for k in range(2):
            nc.sync.dma_start(
                xc[cx][:, k * XB * 128:(k + 1) * XB * 128],
                xT_in[k * 128:(k + 1) * 128,
                      cx * XB * 128:(cx + 1) * XB * 128])

    emit_xc_dma(0)
    att2 = const.tile([128, 2 * C_OUT], F32, name="att2")  # attsrc | attdst
    nc.sync.dma_start(att2[:], att_rep_in[:])
    xl8 = const.tile([128, 2 * JB], F16, name="xl8")      # k0 | k1 halves
    nc.sync.dma_start(
        xl8[:].rearrange("p (two j) -> p two j", two=2),
        xTloc_in[:].rearrange("(two p) j -> p two j", two=2))
    attsrc = att2[:, 0:C_OUT]
    attdst = att2[:, C_OUT:2 * C_OUT]

    adj_tiles = []

    def emit_adj_dma(q, split=False):
        adj_q = adj_pool.tile([128, QUAD * JB], F16, tag="adj",
                              name=f"adj{q}")
        if split:
            half = adj_r[q][:, 0:QUAD // 2, :]
            nc.sync.dma_start(adj_q[:, 0:QUAD * JB // 2], half)
            nc.sync.dma_start(adj_q[:, QUAD * JB // 2:],
                              adj_r[q][:, QUAD // 2:QUAD, :])
        else:
            nc.sync.dma_start(adj_q[:], adj_r[q])
        adj_tiles.append(adj_q)

    # interleave adj and xc chunks; first quad split for an early start
    emit_adj_dma(0, split=True)
    emit_adj_dma(1)
    emit_xc_dma(1)
    emit_adj_dma(2)
    emit_adj_dma(3)
    emit_xc_dma(2)
    emit_adj_dma(4)
    emit_adj_dma(5)
    emit_xc_dma(3)
    emit_adj_dma(6)
    emit_adj_dma(7)
    # adj quads 8..15 emitted in the main loop (pool bufs gate prefetch)

    # ---- device-side constants ----
    ones_col = const.tile([128, 1], F16, name="ones_col")
    nc.vector.memset(ones_col[:], 1.0)
    ones_row = const.tile([1, 128], F32, name="ones_row")
    nc.vector.memset(ones_row[:], 1.0)

    # wsrc[k] = sum_c W[k-block, c] att_src[c]; wdst likewise (DVE STT with
    # free-dim accumulate), then cast into W_sb col / fp8.
    wsrc = const.tile([128, 2], F32, name="wsrc")
    wdst = const.tile([128, 2], F32, name="wdst")
    for k in range(2):
        sc = scratch.tile([128, C_OUT], F32, tag="scr", name=f"scs{k}")
        nc.vector.scalar_tensor_tensor(
            sc[:], W_sb[:, k * 129:k * 129 + 128], 1.0, attsrc,
            op0=ALU.mult, op1=ALU.mult, accum_out=wsrc[:, k:k + 1])
        sc2 = scratch.tile([128, C_OUT], F32, tag="scr", name=f"scd{k}")
        nc.vector.scalar_tensor_tensor(
            sc2[:], W_sb[:, k * 129:k * 129 + 128], 1.0, attdst,
            op0=ALU.mult, op1=ALU.mult, accum_out=wdst[:, k:k + 1])
    for k in range(2):
        nc.vector.tensor_copy(W_sb[:, k * 129 + 128:k * 129 + 129],
                              wsrc[:, k:k + 1])
    wdst8 = const.tile([128, 2], F16, name="wdst8")
    nc.vector.tensor_copy(wdst8[:], wdst[:])

    # ---- a_dst path -> q_rep (all fp8; q_j error cancels columnwise) ----
    adst_row = const.tile([1, JB], F32, name="adst_row")
    for hf in range(2):
        ap = ps_pre.tile([1, 512], F32, tag="adst", name=f"adstp{hf}")
        for k in range(2):
            nc.tensor.matmul(ap[:],
                             lhsT=wdst8[:, k:k + 1],
                             rhs=xl8[:, k * JB + hf * 512:k * JB + (hf + 1) * 512],
                             start=(k == 0), stop=(k == 1))
        nc.scalar.copy(adst_row[0:1, hf * 512:(hf + 1) * 512], ap[:])
    q_rep = const.tile([128, JB], F16, name="q_rep")
    for hf in range(2):
        qp = ps_pre.tile([128, 512], F32, tag="qrep", name=f"qp{hf}")
        nc.tensor.matmul(qp[:], lhsT=ones_row[:],
                         rhs=adst_row[0:1, hf * 512:(hf + 1) * 512],
                         start=True, stop=True)
        nc.scalar.activation(q_rep[:, hf * 512:(hf + 1) * 512], qp[:],
                             ACTF.Exp, scale=0.8)

    # ---- h tiles + a_src (PE matmul w/ fused wsrc col; Pool copies) ----
    h_tiles = []
    asrc_g = [const.tile([128, GRP], F32, tag=f"asrc{g}", name=f"asrc{g}")
              for g in range(NT // GRP)]
    ea_g = [const.tile([128, GRP], F32, tag=f"ea{g}", name=f"ea{g}")
            for g in range(NT // GRP)]   # exp(a_src)
    u_g = [const.tile([128, GRP], F32, tag=f"u{g}", name=f"u{g}")
           for g in range(NT // GRP)]    # exp(0.2 a_src)
    for t in range(NT):
        cx, ti = divmod(t, XB)
        g, gi = divmod(t, GRP)
        hp = ps_h.tile([128, 129], F32, tag="hps", name=f"hps{t}")
        for k in range(2):
            nc.tensor.matmul(
                hp[:],
                lhsT=xc[cx][:, k * XB * 128 + ti * 128:
                            k * XB * 128 + (ti + 1) * 128],
                rhs=W_sb[:, k * 129:(k + 1) * 129],
                start=(k == 0), stop=(k == 1))
        h_t = h_pool.tile([128, 129], F16, tag="h", name=f"h{t}")
        nc.scalar.copy(h_t[:], hp[:])
        nc.gpsimd.tensor_copy(asrc_g[g][:, gi:gi + 1], h_t[:, 128:129])
        h_tiles.append(h_t)
        if gi == GRP - 1:
            nc.scalar.activation(ea_g[g][:], asrc_g[g][:], ACTF.Exp,
                                 scale=1.0)
            nc.scalar.activation(u_g[g][:], asrc_g[g][:], ACTF.Exp,
                                 scale=0.2)

    # ---- main masked-matmul loop (quad granularity) ----
    num_ps = [ps_acc.tile([C_OUT, 512], F32, tag=f"nps{hf}", name=f"nps{hf}")
              for hf in range(2)]
    den_ps = [ps_acc.tile([1, 512], F32, tag=f"dps{hf}", name=f"dps{hf}")
              for hf in range(2)]
    DEN_SHIFT = {4, 8, 12}   # quads whose den reduction runs on DVE
    half = QUAD * JB // 2
    for q in range(NQ):
        if q + 8 < NQ:
            emit_adj_dma(q + 8)
        adj_q = adj_tiles[q]
        t2_q = t2_pool.tile([128, QUAD * JB], F16, tag="t2", name=f"t2_{q}")
        for a in range(QUAD):
            t = q * QUAD + a
            g, gi = divmod(t, GRP)
            nc.vector.tensor_scalar(
                t2_q[:, a * JB:(a + 1) * JB], q_rep[:],
                ea_g[g][:, gi:gi + 1], u_g[g][:, gi:gi + 1],
                op0=ALU.mult, op1=ALU.max)
        m_q = m_pool.tile([128, QUAD * JB], F16, tag="m", name=f"m{q}")
        if q == 0 or q == NQ - 1:
            nc.vector.tensor_tensor(m_q[:, 0:half], t2_q[:, 0:half],
                                    adj_q[:, 0:half], op=ALU.mult)
            nc.vector.tensor_tensor(m_q[:, half:], t2_q[:, half:],
                                    adj_q[:, half:], op=ALU.mult)
        else:
            nc.vector.tensor_tensor(m_q[:], t2_q[:], adj_q[:], op=ALU.mult)
        for a in range(QUAD):
            t = q * QUAD + a
            for hf in range(2):
                ms = m_q[:, a * JB + hf * 512:a * JB + (hf + 1) * 512]
                nc.tensor.matmul(num_ps[hf][:], lhsT=h_tiles[t][:, 0:128],
                                 rhs=ms, start=(t == 0), stop=(t == NT - 1))
                if q not in DEN_SHIFT:
                    nc.tensor.matmul(den_ps[hf][:], lhsT=ones_col[:], rhs=ms,
                                     start=(t == 0), stop=(t == NT - 1))
        if q in DEN_SHIFT:
            # den contribution of this quad: sum the 4 m tiles on DVE, then
            # one PE reduction per half on the summed tile
            s01 = dsum_pool.tile([128, JB], F16, tag="s01", name=f"s01_{q}")
            nc.vector.tensor_tensor(s01[:], m_q[:, 0:JB], m_q[:, JB:2 * JB],
                                    op=ALU.add)
            s23 = dsum_pool.tile([128, JB], F16, tag="s23", name=f"s23_{q}")
            nc.vector.tensor_tensor(s23[:], m_q[:, 2 * JB:3 * JB],
                                    m_q[:, 3 * JB:4 * JB], op=ALU.add)
            s = dsum_pool.tile([128, JB], F16, tag="s", name=f"s_{q}")
            nc.vector.tensor_tensor(s[:], s01[:], s23[:], op=ALU.add)
            for hf in range(2):
                nc.tensor.matmul(den_ps[hf][:], lhsT=ones_col[:],
                                 rhs=s[:, hf * 512:(hf + 1) * 512],
                                 start=False, stop=False,
                                 skip_group_check=True)

    # ---- epilogue ----
    num_sb = const.tile([C_OUT, JB], F16, name="num_sb")
    den_sb = const.tile([1, JB], F32, name="den_sb")
    nc.scalar.copy(num_sb[:, 0:512], num_ps[0][:])
    nc.vector.tensor_copy(num_sb[:, 512:1024], num_ps[1][:])
    nc.scalar.copy(den_sb[0:1, 0:512], den_ps[0][:])
    nc.vector.tensor_copy(den_sb[0:1, 512:1024], den_ps[1][:])
    nc.sync.dma_start(numT_out[:], num_sb[:])
    nc.sync.dma_start(den_out[:], den_sb[:])


def build_nc(reps=1):
    key = ("nc", reps)
    if key in _nc_cache:
        return _nc_cache[key]
    nc = bacc.Bacc("TRN2", target_bir_lowering=False, debug=False,
                   num_devices=NCORES)

    xT_in = nc.dram_tensor("xT", [C_IN, N], F16, kind="ExternalInput")
    xTloc_in = nc.dram_tensor("xTloc", [C_IN, JB], F16, kind="ExternalInput")
    adj_in = nc.dram_tensor("adjc", [N, JB], F16, kind="ExternalInput")
    W_in = nc.dram_tensor("Wt", [128, C_IN], F16, kind="ExternalInput")
    att_rep_in = nc.dram_tensor("att_rep", [128, 2 * C_OUT], F32,
                                kind="ExternalInput")

    numT_out = nc.dram_tensor("numT", [C_OUT, JB], F16, kind="ExternalOutput")
    den_out = nc.dram_tensor("den", [1, JB], F32, kind="ExternalOutput")

    tensors = (xT_in, xTloc_in, adj_in, W_in, att_rep_in,
               numT_out, den_out)

    with tile.TileContext(nc) as tc:
        if reps > 1:
            with tc.For_i(0, reps, 1, hint_engines=(
                    mybir.EngineType.PE, mybir.EngineType.DVE,
                    mybir.EngineType.Activation, mybir.EngineType.SP,
                    mybir.EngineType.Pool)):
                with ExitStack() as ictx:
                    _emit_body(tc, nc, ictx, tensors)
        else:
            with ExitStack() as ctx:
                _emit_body(tc, nc, ctx, tensors)

    nc.compile()
    _nc_cache[key] = nc
    return nc


def make_in_maps(x, adj, W, att_src, att_dst):
    f8 = ml_dtypes.float8_e4m3
    xT = np.ascontiguousarray(x.T.astype(np.float32, copy=False)).astype(
        np.float16)
    Wt = np.ascontiguousarray(
        np.concatenate([W[0:128, :], W[128:256, :]], axis=1)).astype(
        np.float16)
    att_rep = np.ascontiguousarray(np.concatenate([
        np.broadcast_to(att_src.astype(np.float32), (128, C_OUT)),
        np.broadcast_to(att_dst.astype(np.float32), (128, C_OUT))], axis=1))
    in_maps = []
    for d in range(NCORES):
        adj_d = np.ascontiguousarray(
            adj[:, d * JB:(d + 1) * JB].astype(np.float32, copy=False))
        idx = np.arange(JB)
        adj_d[d * JB + idx, idx] = 1.0          # self loops
        adj_d = adj_d.astype(np.float16)        # 0/1: exact
        xTloc = np.ascontiguousarray(xT[:, d * JB:(d + 1) * JB])
        in_maps.append({
            "xT": xT, "xTloc": xTloc, "adjc": adj_d, "Wt": Wt,
            "att_rep": att_rep,
        })
    return in_maps


def postprocess(results, bias):
    blocks = []
    for d in range(NCORES):
        numT = results[d]["numT"].astype(np.float64)   # [C_OUT, JB]
        den = results[d]["den"].astype(np.float64)     # [1, JB]
        blocks.append((numT / den).T)
    out = np.concatenate(blocks, axis=0) + bias.astype(np.float64)[None, :]
    return out.astype(np.float32)


def kernel(x, adj, W, att_src, att_dst, bias):
    nc = build_nc()
    in_maps = make_in_maps(x, adj, W, att_src, att_dst)
    res = run_bass_kernel_spmd(nc, in_maps, list(range(NCORES)))
    kernel._last_result = res
    return postprocess(res.results, bias)


# revision 8
# speedup vs baseline: 1.0388x; 1.0388x over previous
"""DenseGATConv Trainium2 kernel v2 (8 NeuronCores, SPMD, column-sharded).

Same math as v1 (see kernel.py docstring):
    u_i = exp(0.2 a_src_i), e_i = exp(a_src_i), q_j = exp(0.8 a_dst_j)
    M[i,j] = adj[i,j] * max(e_i q_j, u_i)
    out[j,:] = (M^T h)[j,:] / colsum(M)[j] + bias.

v2 engine/schedule changes (driven by timeline-sim engine occupancy):
  - adj DMA'd in 1MB quad-tile chunks interleaved with the xT chunks on the
    SP queue so the adjacency stream is continuous and the tail chunk small.
  - mask-mult (t2 * adj) done as one tensor_tensor per 4-tile span: fewer
    DVE instructions, same 2x DVE mode, lower per-op overhead.
  - h/a_src PSUM->SBUF copies moved from ACT to the idle Pool (gpsimd)
    engine so ACT never serializes the h pipeline (exp groups + q_rep only).
  - a_src fused as a 129th column of the W matmul (no separate PE columns).
  - a_dst path in fp8 (q_j errors cancel between numerator and column sum).
  - numT exported fp16 (den stays f32); halves the output DMA.
"""

import numpy as np
import ml_dtypes
from contextlib import ExitStack

import concourse.bass as bass
import concourse.bacc as bacc
import concourse.tile as tile
from concourse import mybir
from concourse.bass_utils import run_bass_kernel_spmd

F32 = mybir.dt.float32
F16 = mybir.dt.float16
F8 = mybir.dt.float8e4
ALU = mybir.AluOpType
ACTF = mybir.ActivationFunctionType

N, C_IN, C_OUT = 8192, 256, 128
NCORES = 8
JB = N // NCORES          # 1024 destination columns per core
NT = N // 128             # 64 i-tiles
QUAD = 4                  # i-tiles per adj DMA chunk / per mask-mult op
NQ = NT // QUAD           # 16 quads
GRP = 4                   # a_src exp-group size (i-tiles)
XB = 16                   # i-tiles per xT chunk
NXC = NT // XB            # 4 xT chunks

_nc_cache = {}


def _make_pools(tc, ctx):
    return dict(
        const=ctx.enter_context(tc.tile_pool(name="const", bufs=2)),
        xt_pool=ctx.enter_context(tc.tile_pool(name="xt", bufs=5)),
        h_pool=ctx.enter_context(tc.tile_pool(name="h", bufs=96)),
        scratch=ctx.enter_context(tc.tile_pool(name="scr", bufs=2)),
        adj_pool=ctx.enter_context(tc.tile_pool(name="adj", bufs=6)),
        t2_pool=ctx.enter_context(tc.tile_pool(name="t2", bufs=2)),
        m_pool=ctx.enter_context(tc.tile_pool(name="m", bufs=3)),
        dsum_pool=ctx.enter_context(tc.tile_pool(name="dsum", bufs=1)),
        ps_h=ctx.enter_context(tc.tile_pool(name="psh", bufs=2, space="PSUM")),
        ps_acc=ctx.enter_context(tc.tile_pool(name="psacc", bufs=1,
                                              space="PSUM")),
        ps_pre=ctx.enter_context(tc.tile_pool(name="pspre", bufs=1,
                                              space="PSUM")),
    )


def _emit_body(tc, nc, pools, tensors, rep):
    (xT_in, xTloc_in, adj_in, W_in, att_rep_in,
     numT_out, den_out) = tensors

    adj_r = adj_in.rearrange("(c a p) j -> c p a j", a=QUAD, p=128)

    const = pools["const"]
    xt_pool = pools["xt_pool"]
    h_pool = pools["h_pool"]
    scratch = pools["scratch"]
    adj_pool = pools["adj_pool"]
    t2_pool = pools["t2_pool"]
    m_pool = pools["m_pool"]
    dsum_pool = pools["dsum_pool"]
    ps_h = pools["ps_h"]
    ps_acc = pools["ps_acc"]
    ps_pre = pools["ps_pre"]

    # ---- front-loaded DMAs (SP queue, program order == stream order) ----
    # small constants first, then xTloc (a_dst path), then xc0, adj0, xc1,
    # adj1, ... so q_rep and the first h/exp groups are ready when the first
    # adjacency quads land.
    W_sb = const.tile([128, 258], F16, tag="W_sb", name=f"W_sb_{rep}")      # [k*129 .. ] cols,
    W_view = W_sb[:].rearrange("p (two c) -> p two c", two=2)[:, :, 0:128]
    nc.sync.dma_start(W_view, W_in[:].rearrange("p (two c) -> p two c", two=2))

    xc = [xt_pool.tile([128, 2 * XB * 128], F16, tag="xtc", name=f"xc{cx}_{rep}")
          for cx in range(NXC)]

    def emit_xc_dma(cx):
        for k in range(2):
            nc.sync.dma_start(
                xc[cx][:, k * XB * 128:(k + 1) * XB * 128],
                xT_in[k * 128:(k + 1) * 128,
                      cx * XB * 128:(cx + 1) * XB * 128])

    emit_xc_dma(0)
    att2 = const.tile([128, 2 * C_OUT], F32, tag="att2", name=f"att2_{rep}")  # attsrc | attdst
    nc.sync.dma_start(att2[:], att_rep_in[:])
    xl8 = const.tile([128, 2 * JB], F16, tag="xl8", name=f"xl8_{rep}")      # k0 | k1 halves
    nc.sync.dma_start(
        xl8[:].rearrange("p (two j) -> p two j", two=2),
        xTloc_in[:].rearrange("(two p) j -> p two j", two=2))
    attsrc = att2[:, 0:C_OUT]
    attdst = att2[:, C_OUT:2 * C_OUT]

    adj_tiles = []

    def emit_adj_dma(q, split=False):
        adj_q = adj_pool.tile([128, QUAD * JB], F16, tag="adj",
                              name=f"adj{q}_{rep}")
        if split:
            half = adj_r[q][:, 0:QUAD // 2, :]
            nc.sync.dma_start(adj_q[:, 0:QUAD * JB // 2], half)
            nc.sync.dma_start(adj_q[:, QUAD * JB // 2:],
                              adj_r[q][:, QUAD // 2:QUAD, :])
        else:
            nc.sync.dma_start(adj_q[:], adj_r[q])
        adj_tiles.append(adj_q)

    # interleave adj and xc chunks; first quad split for an early start
    emit_adj_dma(0, split=True)
    emit_adj_dma(1)
    emit_xc_dma(1)
    emit_adj_dma(2)
    emit_adj_dma(3)
    emit_xc_dma(2)
    emit_adj_dma(4)
    emit_adj_dma(5)
    emit_xc_dma(3)
    # adj quads 6..15 emitted in the main loop (pool bufs gate prefetch)

    # ---- device-side constants ----
    ones_col = const.tile([128, 1], F16, tag="ones_col", name=f"ones_col_{rep}")
    nc.vector.memset(ones_col[:], 1.0)
    ones_row = const.tile([1, 128], F32, tag="ones_row", name=f"ones_row_{rep}")
    nc.vector.memset(ones_row[:], 1.0)

    # wsrc[k] = sum_c W[k-block, c] att_src[c]; wdst likewise (DVE STT with
    # free-dim accumulate), then cast into W_sb col / fp8.
    wsrc = const.tile([128, 2], F32, tag="wsrc", name=f"wsrc_{rep}")
    wdst = const.tile([128, 2], F32, tag="wdst", name=f"wdst_{rep}")
    for k in range(2):
        sc = scratch.tile([128, C_OUT], F32, tag="scr", name=f"scs{k}_{rep}")
        nc.vector.scalar_tensor_tensor(
            sc[:], W_sb[:, k * 129:k * 129 + 128], 1.0, attsrc,
            op0=ALU.mult, op1=ALU.mult, accum_out=wsrc[:, k:k + 1])
        sc2 = scratch.tile([128, C_OUT], F32, tag="scr", name=f"scd{k}_{rep}")
        nc.vector.scalar_tensor_tensor(
            sc2[:], W_sb[:, k * 129:k * 129 + 128], 1.0, attdst,
            op0=ALU.mult, op1=ALU.mult, accum_out=wdst[:, k:k + 1])
    for k in range(2):
        nc.vector.tensor_copy(W_sb[:, k * 129 + 128:k * 129 + 129],
                              wsrc[:, k:k + 1])
    wdst8 = const.tile([128, 2], F16, tag="wdst8", name=f"wdst8_{rep}")
    nc.vector.tensor_copy(wdst8[:], wdst[:])

    # ---- a_dst path -> q_rep (all fp8; q_j error cancels columnwise) ----
    adst_row = const.tile([1, JB], F32, tag="adst_row", name=f"adst_row_{rep}")
    for hf in range(2):
        ap = ps_pre.tile([1, 512], F32, tag="adst", name=f"adstp{hf}_{rep}")
        for k in range(2):
            nc.tensor.matmul(ap[:],
                             lhsT=wdst8[:, k:k + 1],
                             rhs=xl8[:, k * JB + hf * 512:k * JB + (hf + 1) * 512],
                             start=(k == 0), stop=(k == 1))
        nc.scalar.copy(adst_row[0:1, hf * 512:(hf + 1) * 512], ap[:])
    q_rep = const.tile([128, JB], F16, tag="q_rep", name=f"q_rep_{rep}")
    for hf in range(2):
        qp = ps_pre.tile([128, 512], F32, tag="qrep", name=f"qp{hf}_{rep}")
        nc.tensor.matmul(qp[:], lhsT=ones_row[:],
                         rhs=adst_row[0:1, hf * 512:(hf + 1) * 512],
                         start=True, stop=True)
        nc.scalar.activation(q_rep[:, hf * 512:(hf + 1) * 512], qp[:],
                             ACTF.Exp, scale=0.8)

    # ---- h tiles + a_src (PE matmul w/ fused wsrc col; Pool copies) ----
    h_tiles = []
    asrc_g = [const.tile([128, GRP], F32, tag=f"asrc{g}", name=f"asrc{g}_{rep}")
              for g in range(NT // GRP)]
    ea_g = [const.tile([128, GRP], F32, tag=f"ea{g}", name=f"ea{g}_{rep}")
            for g in range(NT // GRP)]   # exp(a_src)
    u_g = [const.tile([128, GRP], F32, tag=f"u{g}", name=f"u{g}_{rep}")
           for g in range(NT // GRP)]    # exp(0.2 a_src)
    for t in range(NT):
        cx, ti = divmod(t, XB)
        g, gi = divmod(t, GRP)
        hp = ps_h.tile([128, 129], F32, tag="hps", name=f"hps{t}_{rep}")
        for k in range(2):
            nc.tensor.matmul(
                hp[:],
                lhsT=xc[cx][:, k * XB * 128 + ti * 128:
                            k * XB * 128 + (ti + 1) * 128],
                rhs=W_sb[:, k * 129:(k + 1) * 129],
                start=(k == 0), stop=(k == 1))
        h_t = h_pool.tile([128, 129], F16, tag="h", name=f"h{t}_{rep}")
        nc.scalar.copy(h_t[:], hp[:])
        nc.gpsimd.tensor_copy(asrc_g[g][:, gi:gi + 1], h_t[:, 128:129])
        h_tiles.append(h_t)
        if gi == GRP - 1:
            nc.scalar.activation(ea_g[g][:], asrc_g[g][:], ACTF.Exp,
                                 scale=1.0)
            nc.scalar.activation(u_g[g][:], asrc_g[g][:], ACTF.Exp,
                                 scale=0.2)

    # ---- main masked-matmul loop (quad granularity) ----
    num_ps = [ps_acc.tile([C_OUT, 512], F32, tag=f"nps{hf}", name=f"nps{hf}_{rep}")
              for hf in range(2)]
    den_ps = [ps_acc.tile([1, 512], F32, tag=f"dps{hf}", name=f"dps{hf}_{rep}")
              for hf in range(2)]
    DEN_SHIFT = {4, 8, 12}   # quads whose den reduction runs on DVE
    half = QUAD * JB // 2
    for q in range(NQ):
        if q + 6 < NQ:
            emit_adj_dma(q + 6)
        adj_q = adj_tiles[q]
        t2_q = t2_pool.tile([128, QUAD * JB], F16, tag="t2", name=f"t2_{q}_{rep}")
        for a in range(QUAD):
            t = q * QUAD + a
            g, gi = divmod(t, GRP)
            nc.vector.tensor_scalar(
                t2_q[:, a * JB:(a + 1) * JB], q_rep[:],
                ea_g[g][:, gi:gi + 1], u_g[g][:, gi:gi + 1],
                op0=ALU.mult, op1=ALU.max)
        m_q = m_pool.tile([128, QUAD * JB], F16, tag="m", name=f"m{q}_{rep}")
        if q == 0 or q == NQ - 1:
            nc.vector.tensor_tensor(m_q[:, 0:half], t2_q[:, 0:half],
                                    adj_q[:, 0:half], op=ALU.mult)
            nc.vector.tensor_tensor(m_q[:, half:], t2_q[:, half:],
                                    adj_q[:, half:], op=ALU.mult)
        else:
            nc.vector.tensor_tensor(m_q[:], t2_q[:], adj_q[:], op=ALU.mult)
        for a in range(QUAD):
            t = q * QUAD + a
            for hf in range(2):
                ms = m_q[:, a * JB + hf * 512:a * JB + (hf + 1) * 512]
                nc.tensor.matmul(num_ps[hf][:], lhsT=h_tiles[t][:, 0:128],
                                 rhs=ms, start=(t == 0), stop=(t == NT - 1))
                if q not in DEN_SHIFT:
                    nc.tensor.matmul(den_ps[hf][:], lhsT=ones_col[:], rhs=ms,
                                     start=(t == 0), stop=(t == NT - 1))
        if q in DEN_SHIFT:
            # den contribution of this quad: sum the 4 m tiles on DVE, then
            # one PE reduction per half on the summed tile
            s01 = dsum_pool.tile([128, JB], F16, tag="s01", name=f"s01_{q}_{rep}")
            nc.vector.tensor_tensor(s01[:], m_q[:, 0:JB], m_q[:, JB:2 * JB],
                                    op=ALU.add)
            s23 = dsum_pool.tile([128, JB], F16, tag="s23", name=f"s23_{q}_{rep}")
            nc.vector.tensor_tensor(s23[:], m_q[:, 2 * JB:3 * JB],
                                    m_q[:, 3 * JB:4 * JB], op=ALU.add)
            s = dsum_pool.tile([128, JB], F16, tag="s", name=f"s_{q}_{rep}")
            nc.vector.tensor_tensor(s[:], s01[:], s23[:], op=ALU.add)
            for hf in range(2):
                nc.tensor.matmul(den_ps[hf][:], lhsT=ones_col[:],
                                 rhs=s[:, hf * 512:(hf + 1) * 512],
                                 start=False, stop=False,
                                 skip_group_check=True)

    # ---- epilogue ----
    num_sb = const.tile([C_OUT, JB], F16, tag="num_sb", name=f"num_sb_{rep}")
    den_sb = const.tile([1, JB], F32, tag="den_sb", name=f"den_sb_{rep}")
    nc.scalar.copy(num_sb[:, 0:512], num_ps[0][:])
    nc.vector.tensor_copy(num_sb[:, 512:1024], num_ps[1][:])
    nc.scalar.copy(den_sb[0:1, 0:512], den_ps[0][:])
    nc.vector.tensor_copy(den_sb[0:1, 512:1024], den_ps[1][:])
    nc.sync.dma_start(numT_out[:], num_sb[:])
    nc.sync.dma_start(den_out[:], den_sb[:])


def build_nc(reps=1):
    key = ("nc", reps)
    if key in _nc_cache:
        return _nc_cache[key]
    nc = bacc.Bacc("TRN2", target_bir_lowering=False, debug=False,
                   num_devices=NCORES)

    xT_in = nc.dram_tensor("xT", [C_IN, N], F16, kind="ExternalInput")
    xTloc_in = nc.dram_tensor("xTloc", [C_IN, JB], F16, kind="ExternalInput")
    adj_in = nc.dram_tensor("adjc", [N, JB], F16, kind="ExternalInput")
    W_in = nc.dram_tensor("Wt", [128, C_IN], F16, kind="ExternalInput")
    att_rep_in = nc.dram_tensor("att_rep", [128, 2 * C_OUT], F32,
                                kind="ExternalInput")

    numT_out = nc.dram_tensor("numT", [C_OUT, JB], F16, kind="ExternalOutput")
    den_out = nc.dram_tensor("den", [1, JB], F32, kind="ExternalOutput")

    tensors = (xT_in, xTloc_in, adj_in, W_in, att_rep_in,
               numT_out, den_out)

    UNROLL = 2
    with tile.TileContext(nc) as tc:
        with ExitStack() as pctx:
            pools = _make_pools(tc, pctx)
            if reps >= 2 * UNROLL:
                n_loop, n_rem = divmod(reps, UNROLL)
                with tc.For_i(0, n_loop, 1, hint_engines=(
                        mybir.EngineType.PE, mybir.EngineType.DVE,
                        mybir.EngineType.Activation, mybir.EngineType.SP,
                        mybir.EngineType.Pool)):
                    for r in range(UNROLL):
                        _emit_body(tc, nc, pools, tensors, r)
                for r in range(n_rem):
                    _emit_body(tc, nc, pools, tensors, UNROLL + r)
            else:
                for r in range(reps):
                    _emit_body(tc, nc, pools, tensors, r)

    nc.compile()
    _nc_cache[key] = nc
    return nc


def make_in_maps(x, adj, W, att_src, att_dst):
    f8 = ml_dtypes.float8_e4m3
    xT = np.ascontiguousarray(x.T.astype(np.float32, copy=False)).astype(
        np.float16)
    Wt = np.ascontiguousarray(
        np.concatenate([W[0:128, :], W[128:256, :]], axis=1)).astype(
        np.float16)
    att_rep = np.ascontiguousarray(np.concatenate([
        np.broadcast_to(att_src.astype(np.float32), (128, C_OUT)),
        np.broadcast_to(att_dst.astype(np.float32), (128, C_OUT))], axis=1))
    in_maps = []
    for d in range(NCORES):
        adj_d = np.ascontiguousarray(
            adj[:, d * JB:(d + 1) * JB].astype(np.float32, copy=False))
        idx = np.arange(JB)
        adj_d[d * JB + idx, idx] = 1.0          # self loops
        adj_d = adj_d.astype(np.float16)        # 0/1: exact
        xTloc = np.ascontiguousarray(xT[:, d * JB:(d + 1) * JB])
        in_maps.append({
            "xT": xT, "xTloc": xTloc, "adjc": adj_d, "Wt": Wt,
            "att_rep": att_rep,
        })
    return in_maps


def postprocess(results, bias):
    blocks = []
    for d in range(NCORES):
        numT = results[d]["numT"].astype(np.float64)   # [C_OUT, JB]
        den = results[d]["den"].astype(np.float64)     # [1, JB]
        blocks.append((numT / den).T)
    out = np.concatenate(blocks, axis=0) + bias.astype(np.float64)[None, :]
    return out.astype(np.float32)


def kernel(x, adj, W, att_src, att_dst, bias):
    nc = build_nc()
    in_maps = make_in_maps(x, adj, W, att_src, att_dst)
    res = run_bass_kernel_spmd(nc, in_maps, list(range(NCORES)))
    kernel._last_result = res
    return postprocess(res.results, bias)
